# revision 1
# baseline (speedup 1.0000x reference)
"""LlamaAttention forward on 8 Trainium2 NeuronCores (tensor-parallel over heads).

Sharding: heads sharded 4-per-core for QKV + attention; attention outputs
AllGather'd (bf16, feature-major-transposed layout), out-projection sharded
over output features; host concatenates the 8 output-feature shards.

Layout strategy (no on-chip transposes of q/k, no cross-partition ops):
  - host uploads x pre-transposed (xT[hid, tok], bf16)
  - qT/kT computed as [hd, tok] via matmul(lhsT=WqT_chunk, rhs=xT_chunk)
  - RoPE uses a de-interleaving feature permutation folded into Wq/Wk rows on
    the host, making rotate_half a partition rotation by 64, implemented as a
    matmul with a constant 128x128 permutation matrix P
  - scoresT[k, q] = matmul(lhsT=kT_tile, rhs=qT); softmax runs max-free
    (scores are O(5) here), masking via multiplicative exp(mask) on the
    affected 128x128 blocks only; fully-masked blocks are skipped entirely
  - probsT used as lhsT against v_aug=[v | ones] (natural [tok, hd] layout)
    giving av[q, hd] plus the softmax denominator as column 128 for free
  - per-token 1/denom applied via per-partition activation scale, attn tiles
    transposed on the PE into [hd, tok] and DMA'd to the AllGather buffer
"""
import sys
import math

sys.path.insert(0, "/opt/trn_rl_repo")

import numpy as np
import ml_dtypes

B, S, HID, NH, HD = 2, 1024, 4096, 32, 128
NCORES = 8
HPC = NH // NCORES          # 4 heads per core
FS = HPC * HD               # 512 features per shard
T = B * S                   # 2048 tokens
NKT = S // 128              # 8 key tiles per batch
NC_HID = HID // 128         # 32 contraction chunks
VW = 133                    # per-head stride in v tile (128 feats + 1 ones + pad)

_bf16 = ml_dtypes.bfloat16

_cache = {}


def _host_prep(x, Wq, bq, Wk, bk, Wv, bv, Wo, bo, position_ids, attention_mask):
    perm = np.concatenate([np.arange(0, HD, 2), np.arange(1, HD, 2)])  # de-interleave
    scale = 1.0 / math.sqrt(HD)

    # mask block structure
    em = np.exp(attention_mask[0, 0].astype(np.float64)).astype(np.float32)
    emT = em.T  # [k, q]
    compute_q = []   # per j: list of q-blocks to compute
    needs_mul = []   # per j: list of q-blocks needing exp(mask) multiply
    for j in range(NKT):
        cq, nm = [], []
        for qb in range(S // 128):
            blk = emT[128 * j:128 * j + 128, 128 * qb:128 * qb + 128]
            if (blk == 0.0).all():
                continue
            cq.append(qb)
            if not (blk == 1.0).all():
                nm.append(qb)
        compute_q.append(cq)
        needs_mul.append(nm)
    av_js = [[j for j in range(NKT) if i in compute_q[j]] for i in range(S // 128)]
    mask_key = (tuple(tuple(c) for c in compute_q), tuple(tuple(n) for n in needs_mul))

    # RoPE tables, transposed+de-interleaved+sign-folded: [B, 128, S]
    inv_freq = 1.0 / (10000.0 ** (np.arange(0, HD, 2, dtype=np.float32) / HD))
    ang = position_ids.astype(np.float32)[:, None, :] * inv_freq[None, :, None]  # [B,64,S]
    cosT = np.concatenate([np.cos(ang), np.cos(ang)], axis=1)                    # [B,128,S]
    sinT = np.concatenate([-np.sin(ang), np.sin(ang)], axis=1)                   # sign folded

    pmat = np.zeros((HD, HD), np.float32)
    for i in range(HD):
        pmat[i, (i + 64) % HD] = 1.0
    ident = np.eye(128, dtype=np.float32)

    Wq_p = (Wq.reshape(NH, HD, HID)[:, perm, :] * scale).reshape(NH, HD, HID)
    Wk_p = Wk.reshape(NH, HD, HID)[:, perm, :].reshape(NH, HD, HID)
    bq_p = (bq.reshape(NH, HD)[:, perm] * scale)
    bk_p = bk.reshape(NH, HD)[:, perm]

    shared = {
        "xT": np.ascontiguousarray(x.reshape(T, HID).T).astype(_bf16),
        "pmat": pmat.astype(_bf16),
        "ident": ident.astype(_bf16),
        "cosT": cosT.astype(_bf16),
        "sinT": sinT.astype(_bf16),
        "emT": np.ascontiguousarray(emT.reshape(NKT, 128, S)).astype(_bf16),
    }
    per_core = []
    for c in range(NCORES):
        hs = slice(c * HPC, (c + 1) * HPC)
        wq = Wq_p[hs].reshape(FS, HID)
        wk = Wk_p[hs].reshape(FS, HID)
        wv = Wv[c * FS:(c + 1) * FS, :]
        wo = Wo[c * FS:(c + 1) * FS, :]
        per_core.append({
            "wqT": np.ascontiguousarray(wq.T).astype(_bf16),
            "wkT": np.ascontiguousarray(wk.T).astype(_bf16),
            "wvT": np.ascontiguousarray(wv.T).astype(_bf16),
            "woT": np.ascontiguousarray(wo.T).astype(_bf16),
            "biasq": np.ascontiguousarray(bq_p[hs].T).astype(np.float32),  # [128, 4]
            "biask": np.ascontiguousarray(bk_p[hs].T).astype(np.float32),
        })
    return shared, per_core, (compute_q, needs_mul, av_js), mask_key


# input order for the run_kernel pytree (list per core)
_IN_NAMES = ["xT", "wqT", "wkT", "wvT", "woT", "pmat", "ident",
             "cosT", "sinT", "emT", "biasq", "biask"]


def _build_kernel_fn(mask_info):
    import concourse.tile as tile
    from concourse import mybir
    dt = mybir.dt
    AF = mybir.ActivationFunctionType
    compute_q, needs_mul, av_js = mask_info

    def kern(tc, outs, ins):
        nc = tc.nc
        (xT_h, wqT_h, wkT_h, wvT_h, woT_h, pmat_h, ident_h,
         cosT_h, sinT_h, emT_h, biasq_h, biask_h) = ins
        out_h = outs[0]

        from contextlib import ExitStack
        with ExitStack() as ctx:
            E = ctx.enter_context
            cpool = E(tc.tile_pool(name="const", bufs=1))
            xpool = E(tc.tile_pool(name="x", bufs=1))
            wpool = E(tc.tile_pool(name="w", bufs=3))
            spool = E(tc.tile_pool(name="s", bufs=2))
            qkpool = E(tc.tile_pool(name="qk", bufs=1))
            vpool = E(tc.tile_pool(name="v", bufs=1))
            prpool = E(tc.tile_pool(name="pr", bufs=1))
            appool = E(tc.tile_pool(name="ap", bufs=2))
            opool = E(tc.tile_pool(name="o", bufs=2))
            pspool = E(tc.tile_pool(name="ps", bufs=1, space="PSUM"))
            ps2pool = E(tc.tile_pool(name="ps2", bufs=1, space="PSUM"))
            dpool = E(tc.tile_pool(name="dram", bufs=1, space="DRAM"))

            # constants
            pm = cpool.tile([128, 128], dt.bfloat16, tag="pmat", name="pmat")
            nc.sync.dma_start(pm[:], pmat_h[:, :])
            idn = cpool.tile([128, 128], dt.bfloat16, tag="ident", name="ident")
            nc.sync.dma_start(idn[:], ident_h[:, :])
            bq_sb = cpool.tile([128, HPC], dt.float32, tag="biasq", name="biasq")
            nc.sync.dma_start(bq_sb[:], biasq_h[:, :])
            bk_sb = cpool.tile([128, HPC], dt.float32, tag="biask", name="biask")
            nc.sync.dma_start(bk_sb[:], biask_h[:, :])
            cos_sb, sin_sb, em_sb = [], [], []
            for b in range(B):
                t_ = cpool.tile([128, S], dt.bfloat16, tag=f"cos{b}", name=f"cos{b}")
                nc.sync.dma_start(t_[:], cosT_h[b])
                cos_sb.append(t_)
                t_ = cpool.tile([128, S], dt.bfloat16, tag=f"sin{b}", name=f"sin{b}")
                nc.sync.dma_start(t_[:], sinT_h[b])
                sin_sb.append(t_)
            for j in range(NKT):
                t_ = cpool.tile([128, S], dt.bfloat16, tag=f"em{j}", name=f"em{j}")
                nc.sync.dma_start(t_[:], emT_h[j])
                em_sb.append(t_)

            attn_sh = dpool.tile([FS, T], dt.bfloat16, tag="attn_sh", name="attn_sh")
            ag_out = dpool.tile([NCORES * FS, T], dt.bfloat16, tag="ag_out", name="ag_out",
                                addr_space="Shared")

            for b in range(B):
                # resident xT chunk tiles for this batch
                xt = []
                for c in range(NC_HID):
                    t_ = xpool.tile([128, S], dt.bfloat16, tag=f"x{c}", name=f"x{c}")
                    nc.sync.dma_start(t_[:], xT_h[128 * c:128 * c + 128,
                                                 S * b:S * b + S])
                    xt.append(t_)

                # ---- Q and K projections (transposed layout [hd, tok]) ----
                qr, kr = [], []
                for proj, (wT_h, bias_sb, dest) in enumerate(
                        [(wqT_h, bq_sb, qr), (wkT_h, bk_sb, kr)]):
                  for hg in range(HPC // 2):
                    hpair = [2 * hg, 2 * hg + 1]
                    psq = {h: [pspool.tile([128, 512], dt.float32, tag=f"pb{2*hi+hf}", name=f"pb{2*hi+hf}")
                            for hf in range(2)] for hi, h in enumerate(hpair)}
                    for c in range(NC_HID):
                        wt = wpool.tile([128, 256], dt.bfloat16, tag="wt", name="wt")
                        nc.sync.dma_start(wt[:], wT_h[128 * c:128 * c + 128,
                                                      256 * hg:256 * hg + 256])
                        for hi, h in enumerate(hpair):
                            lhs = wt[:, 128 * hi:128 * hi + 128]
                            nc.tensor.matmul(psq[h][0][:], lhs, xt[c][:, 0:512],
                                             start=(c == 0), stop=(c == NC_HID - 1))
                            nc.tensor.matmul(psq[h][1][:], lhs, xt[c][:, 512:1024],
                                             start=(c == 0), stop=(c == NC_HID - 1))
                    for h in hpair:
                        u = spool.tile([128, S], dt.bfloat16, tag="u", name="u")
                        nc.scalar.activation(u[:, 0:512], psq[h][0][:], AF.Identity,
                                             bias=bias_sb[:, h:h + 1])
                        nc.scalar.activation(u[:, 512:1024], psq[h][1][:], AF.Identity,
                                             bias=bias_sb[:, h:h + 1])
                        psr0 = ps2pool.tile([128, 512], dt.float32, tag="psr", name="psr")
                        nc.tensor.matmul(psr0[:], pm[:], u[:, 0:512],
                                         start=True, stop=True)
                        psr1 = ps2pool.tile([128, 512], dt.float32, tag="psr", name="psr")
                        nc.tensor.matmul(psr1[:], pm[:], u[:, 512:1024],
                                         start=True, stop=True)
                        rot = spool.tile([128, S], dt.bfloat16, tag="rot", name="rot")
                        nc.scalar.copy(rot[:, 0:512], psr0[:])
                        nc.scalar.copy(rot[:, 512:1024], psr1[:])
                        t1 = spool.tile([128, S], dt.bfloat16, tag="t1", name="t1")
                        nc.vector.tensor_mul(t1[:], u[:], cos_sb[b][:])
                        dst = qkpool.tile([128, S], dt.bfloat16,
                                          tag=f"{'qk'[proj]}r{h}", name=f"{'qk'[proj]}r{h}")
                        nc.vector.tensor_mul(dst[:], rot[:], sin_sb[b][:])
                        nc.vector.tensor_add(dst[:], dst[:], t1[:])
                        dest.append(dst)

                # ---- V projection (natural layout [tok, feat]) ----
                vt = []
                for t in range(NKT):
                    t_ = vpool.tile([128, HPC * VW], dt.bfloat16, tag=f"v{t}", name=f"v{t}")
                    vt.append(t_)
                for tg in range(2):
                    psv = {t: pspool.tile([128, FS], dt.float32, tag=f"pb{t-4*tg}", name=f"pb{t-4*tg}")
                           for t in range(4 * tg, 4 * tg + 4)}
                    for c in range(NC_HID):
                        wt = wpool.tile([128, FS], dt.bfloat16, tag="wtv", name="wtv")
                        nc.sync.dma_start(wt[:], wvT_h[128 * c:128 * c + 128, :])
                        for t in range(4 * tg, 4 * tg + 4):
                            nc.tensor.matmul(psv[t][:], xt[c][:, 128 * t:128 * t + 128],
                                             wt[:], start=(c == 0), stop=(c == NC_HID - 1))
                    for t in range(4 * tg, 4 * tg + 4):
                        for h in range(HPC):
                            nc.scalar.copy(vt[t][:, VW * h:VW * h + 128],
                                           psv[t][:, 128 * h:128 * h + 128])
                            nc.vector.memset(vt[t][:, VW * h + 128:VW * h + 129], 1.0)

                # ---- attention per head ----
                for h in range(HPC):
                    pj = []
                    for j in range(NKT):
                        if not compute_q[j]:
                            pj.append(None)
                            continue
                        p_ = prpool.tile([128, S], dt.bfloat16, tag=f"p{j}", name=f"p{j}")
                        pj.append(p_)
                        # contiguous q span, in <=512 chunks
                        q_lo = 128 * compute_q[j][0]
                        q_hi = 128 * (compute_q[j][-1] + 1)
                        cs = q_lo
                        while cs < q_hi:
                            ce = min(cs + 512, q_hi)
                            pss = ps2pool.tile([128, 512], dt.float32, tag="ps_s", name="ps_s")
                            nc.tensor.matmul(pss[:, 0:ce - cs],
                                             kr[h][:, 128 * j:128 * j + 128],
                                             qr[h][:, cs:ce], start=True, stop=True)
                            nc.scalar.activation(p_[:, cs:ce], pss[:, 0:ce - cs],
                                                 AF.Exp)
                            cs = ce
                        for qb in needs_mul[j]:
                            nc.vector.tensor_mul(p_[:, 128 * qb:128 * qb + 128],
                                                 p_[:, 128 * qb:128 * qb + 128],
                                                 em_sb[j][:, 128 * qb:128 * qb + 128])
                    attnT = appool.tile([128, S], dt.bfloat16, tag="attnT", name="attnT")
                    for i in range(S // 128):
                        js = av_js[i]
                        psav = ps2pool.tile([128, 132], dt.float32, tag="ps_av", name="ps_av")
                        for jx, j in enumerate(js):
                            nc.tensor.matmul(psav[:, 0:129],
                                             pj[j][:, 128 * i:128 * i + 128],
                                             vt[j][:, VW * h:VW * h + 129],
                                             start=(jx == 0), stop=(jx == len(js) - 1))
                        rc = opool.tile([128, 1], dt.float32, tag="rc", name="rc")
                        nc.vector.reciprocal(rc[:], psav[:, 128:129])
                        an = opool.tile([128, 128], dt.bfloat16, tag="an", name="an")
                        nc.scalar.mul(an[:], psav[:, 0:128], rc[:])
                        pst = ps2pool.tile([128, 128], dt.bfloat16, tag="ps_tr", name="ps_tr")
                        nc.tensor.transpose(pst[:], an[:], idn[:])
                        nc.vector.tensor_copy(attnT[:, 128 * i:128 * i + 128], pst[:])
                    nc.sync.dma_start(
                        attn_sh[128 * h:128 * h + 128, S * b:S * b + S], attnT[:])

            # ---- AllGather attention shards ----
            nc.gpsimd.collective_compute(
                "AllGather", mybir.AluOpType.bypass,
                replica_groups=[list(range(NCORES))],
                ins=[attn_sh.opt()], outs=[ag_out.opt()],
            )

            # ---- output projection (out features sharded) ----
            for quarter in range(4):
                pso = {t: pspool.tile([128, FS], dt.float32, tag=f"pb{t}", name=f"pb{t}")
                       for t in range(4)}
                for c in range(NC_HID):
                    agt = spool.tile([128, 512], dt.bfloat16, tag="ag", name="ag")
                    nc.sync.dma_start(agt[:], ag_out[128 * c:128 * c + 128,
                                                     512 * quarter:512 * quarter + 512])
                    wt = wpool.tile([128, FS], dt.bfloat16, tag="wtv", name="wtv")
                    nc.sync.dma_start(wt[:], woT_h[128 * c:128 * c + 128, :])
                    for t in range(4):
                        nc.tensor.matmul(pso[t][:], agt[:, 128 * t:128 * t + 128],
                                         wt[:], start=(c == 0), stop=(c == NC_HID - 1))
                for t in range(4):
                    osb = opool.tile([128, FS], dt.float32, tag="osb", name="osb")
                    nc.scalar.copy(osb[:], pso[t][:])
                    nc.sync.dma_start(
                        out_h[512 * quarter + 128 * t:512 * quarter + 128 * t + 128, :],
                        osb[:])

    return kern


def _get_program(mask_key, mask_info, shapes):
    if mask_key in _cache:
        return _cache[mask_key]
    import os
    import concourse.tile as tile
    from concourse import bacc, mybir

    trace_sim = bool(os.environ.get("KBENCH_TRACE_SIM"))
    nc = bacc.Bacc("TRN2", target_bir_lowering=False, debug=False,
                   enable_asserts=True, num_devices=NCORES)
    in_aps = []
    for n in _IN_NAMES:
        arr_shape, arr_dt = shapes[n]
        in_aps.append(nc.dram_tensor(
            "in_" + n, list(arr_shape), mybir.dt.from_np(np.dtype(arr_dt)),
            kind="ExternalInput").ap())
    out_ap = nc.dram_tensor("out_sh", [T, FS], mybir.dt.float32,
                            kind="ExternalOutput").ap()
    kern = _build_kernel_fn(mask_info)
    with tile.TileContext(nc, trace_sim=trace_sim) as tc:
        kern(tc, [out_ap], in_aps)
    nc.compile()
    _cache[mask_key] = nc
    return nc


def kernel(x, Wq, bq, Wk, bk, Wv, bv, Wo, bo, position_ids, attention_mask):
    x = np.asarray(x, dtype=np.float32)
    Wq, bq = np.asarray(Wq, np.float32), np.asarray(bq, np.float32)
    Wk, bk = np.asarray(Wk, np.float32), np.asarray(bk, np.float32)
    Wv, bv = np.asarray(Wv, np.float32), np.asarray(bv, np.float32)
    Wo, bo = np.asarray(Wo, np.float32), np.asarray(bo, np.float32)
    position_ids = np.asarray(position_ids)
    attention_mask = np.asarray(attention_mask, np.float32)

    shared, per_core, mask_info, mask_key = _host_prep(
        x, Wq, bq, Wk, bk, Wv, bv, Wo, bo, position_ids, attention_mask)

    m0 = {**shared, **per_core[0]}
    shapes = {n: (m0[n].shape, m0[n].dtype) for n in _IN_NAMES}
    nc = _get_program(mask_key, mask_info, shapes)

    from concourse import bass2jax
    in_maps = [{"in_" + n: {**shared, **per_core[c]}[n] for n in _IN_NAMES}
               for c in range(NCORES)]
    results = bass2jax.run_bass_via_pjrt(nc, in_maps, n_cores=NCORES)
    out = np.concatenate([results[c]["out_sh"] for c in range(NCORES)], axis=1)

    kernel._last_in_maps = in_maps
    kernel._last_nc = nc

    out = out + (bv @ Wo.T) + bo            # host-folded v/out biases
    return out.reshape(B, S, HID).astype(np.float32)


def bench(iters=10):
    """Time repeated executions of the last-built program via PJRT.

    Returns (best_ns, avg_ns) per iteration. Must be called after kernel().
    """
    import time
    import jax
    import jax.numpy as jnp
    from jax.sharding import Mesh, PartitionSpec
    from concourse import bass2jax, mybir
    from jax.experimental.shard_map import shard_map

    nc = kernel._last_nc
    in_maps = kernel._last_in_maps
    bass2jax.install_neuronx_cc_hook()

    in_names, out_names, out_avals, zero_outs = [], [], [], []
    partition_name = nc.partition_id_tensor.name if nc.partition_id_tensor else None
    for alloc in nc.m.functions[0].allocations:
        import concourse.mybir as mb
        if not isinstance(alloc, mb.MemoryLocationSet):
            continue
        name = alloc.memorylocations[0].name
        if alloc.kind == "ExternalInput":
            if name != partition_name:
                in_names.append(name)
        elif alloc.kind == "ExternalOutput":
            shape = tuple(alloc.tensor_shape)
            dtype = mb.dt.np(alloc.dtype)
            out_names.append(name)
            out_avals.append(jax.core.ShapedArray(shape, dtype))
            zero_outs.append(np.zeros(shape, dtype))
    n_params = len(in_names)
    all_in_names = in_names + out_names
    if partition_name is not None:
        all_in_names.append(partition_name)

    def _body(*args):
        operands = list(args)
        if partition_name is not None:
            operands.append(bass2jax.partition_id_tensor())
        outs = bass2jax._bass_exec_p.bind(
            *operands,
            out_avals=tuple(out_avals),
            in_names=tuple(all_in_names),
            out_names=tuple(out_names),
            lowering_input_output_aliases=(),
            sim_require_finite=True,
            sim_require_nnan=True,
            nc=nc,
        )
        return tuple(outs)

    devices = jax.devices()[:NCORES]
    mesh = Mesh(np.asarray(devices), ("core",))
    n_outs = len(out_names)
    in_specs = (PartitionSpec("core"),) * (n_params + n_outs)
    out_specs = (PartitionSpec("core"),) * n_outs
    sharded = jax.jit(shard_map(_body, mesh=mesh, in_specs=in_specs,
                                out_specs=out_specs, check_rep=False),
                      keep_unused=True)
    concat_in = [np.concatenate([np.asarray(in_maps[c][nme]) for c in range(NCORES)],
                                axis=0) for nme in in_names]
    concat_zeros = [np.zeros((NCORES * z.shape[0], *z.shape[1:]), z.dtype)
                    for z in zero_outs]
    from jax.sharding import NamedSharding
    shardings = [NamedSharding(mesh, PartitionSpec("core"))] * (n_params + n_outs)
    dev_in = [jax.device_put(a, s) for a, s in zip(concat_in + concat_zeros, shardings)]
    # warmup (compile)
    out = sharded(*dev_in)
    jax.block_until_ready(out)
    times = []
    for _ in range(3):
        t0 = time.perf_counter()
        outs = [sharded(*dev_in) for _ in range(iters)]
        jax.block_until_ready(outs)
        t1 = time.perf_counter()
        times.append((t1 - t0) / iters)
    return min(times) * 1e9, (sum(times) / len(times)) * 1e9



# revision 34
# speedup vs baseline: 7.0423x; 7.0423x over previous
"""LlamaAttention forward on 8 Trainium2 NeuronCores (DP=2 over batch x TP=4 over heads).

Sharding: cores 0-3 handle batch 0, cores 4-7 batch 1. Within each group of 4,
heads are sharded 8-per-core for QKV + attention; attention outputs are
AllGather'd within the group in two chunks (heads 0-3, then 4-7) so the first
collective overlaps the second half of attention and the output projection
starts on chunk 1 while chunk 2 is still in flight. Output features sharded
4-ways within each group; host assembles the 8 [1024 tok, 1024 feat] shards.

Per-core layout strategy (no cross-partition ops, no on-chip transposes of q/k):
  - host uploads x pre-transposed (xT[hid, tok], bf16), one batch per group
  - qT/kT computed as [hd, tok] via matmul(lhsT=WqT_chunk, rhs=xT_chunk)
  - RoPE uses a de-interleaving feature permutation folded into Wq/Wk rows on
    the host, making rotate_half a partition rotation by 64, implemented as a
    matmul with a constant 128x128 permutation matrix P
  - scoresT[k, q] = matmul(lhsT=kT_tile, rhs=qT); softmax runs max-free
    (scores are O(5) here), masking via multiplicative exp(mask) on the
    affected 128x128 blocks only; fully-masked blocks are skipped entirely
  - probsT used as lhsT against v_aug=[v | ones] (natural [tok, hd] layout)
    giving av[q, hd] plus the softmax denominator as column 128 for free
  - per-token 1/denom applied via per-partition activation scale, attn tiles
    transposed on the PE into [hd, tok] and DMA'd to the AllGather buffers
  - out-projection weight rows reordered on host to match concat(ag1, ag2)
"""
import sys
import math

sys.path.insert(0, "/opt/trn_rl_repo")

import numpy as np
import ml_dtypes

B, S, HID, NH, HD = 2, 1024, 4096, 32, 128
NCORES = 8
TPG = 4                     # cores per tensor-parallel group
HPC = NH // TPG             # 8 heads per core
FS = HPC * HD               # 1024 attn features per core
OFS = HID // TPG            # 1024 output features per core
NKT = S // 128              # 8 key tiles
NC_HID = HID // 128         # 32 contraction chunks
VW = 133                    # per-head stride in v tile (128 feats + 1 ones + pad)

_bf16 = ml_dtypes.bfloat16

_cache = {}


def _host_prep(x, Wq, bq, Wk, bk, Wv, bv, Wo, bo, position_ids, attention_mask):
    perm = np.concatenate([np.arange(0, HD, 2), np.arange(1, HD, 2)])  # de-interleave
    scale = 1.0 / math.sqrt(HD)

    # mask block structure
    em = np.exp(attention_mask[0, 0].astype(np.float64)).astype(np.float32)
    emT = em.T  # [k, q]
    compute_q = []   # per j: list of q-blocks to compute
    needs_mul = []   # per j: list of q-blocks needing exp(mask) multiply
    for j in range(NKT):
        cq, nm = [], []
        for qb in range(S // 128):
            blk = emT[128 * j:128 * j + 128, 128 * qb:128 * qb + 128]
            if (blk == 0.0).all():
                continue
            cq.append(qb)
            if not (blk == 1.0).all():
                nm.append(qb)
        compute_q.append(cq)
        needs_mul.append(nm)
    av_js = [[j for j in range(NKT) if i in compute_q[j]] for i in range(S // 128)]
    mask_key = (tuple(tuple(c) for c in compute_q), tuple(tuple(n) for n in needs_mul))

    # RoPE tables, transposed+de-interleaved+sign-folded: [B, 128, S]
    inv_freq = 1.0 / (10000.0 ** (np.arange(0, HD, 2, dtype=np.float32) / HD))
    ang = position_ids.astype(np.float32)[:, None, :] * inv_freq[None, :, None]  # [B,64,S]
    cosT = np.concatenate([np.cos(ang), np.cos(ang)], axis=1)                    # [B,128,S]
    sinT = np.concatenate([-np.sin(ang), np.sin(ang)], axis=1)                   # sign folded

    pmat = np.zeros((HD, HD), np.float32)
    for i in range(HD):
        pmat[i, (i + 64) % HD] = 1.0
    ident = np.eye(128, dtype=np.float32)

    Wq_p = (Wq.reshape(NH, HD, HID)[:, perm, :] * scale).reshape(NH, HD, HID)
    Wk_p = Wk.reshape(NH, HD, HID)[:, perm, :].reshape(NH, HD, HID)
    bq_p = (bq.reshape(NH, HD)[:, perm] * scale)
    bk_p = bk.reshape(NH, HD)[:, perm]

    shared = {
        "pmat": pmat.astype(_bf16),
        "ident": ident.astype(_bf16),
        "emT": np.ascontiguousarray(emT.reshape(NKT, 128, S)).astype(_bf16),
    }
    per_core = []
    for c in range(NCORES):
        b, r = c // TPG, c % TPG
        hs = slice(HPC * r, HPC * r + HPC)
        wq = Wq_p[hs].reshape(FS, HID)
        wk = Wk_p[hs].reshape(FS, HID)
        wv = Wv[FS * r:FS * r + FS, :]
        woT = Wo[OFS * r:OFS * r + OFS, :].T            # [4096 attn feats, 1024 of]
        order = np.concatenate(
            [np.arange(128) + 128 * (HPC * rr + ll)
             for half in (0, 1) for rr in range(TPG)
             for ll in range(4 * half, 4 * half + 4)])
        woT_ro = np.ascontiguousarray(woT[order, :])
        # hg-major [4, 4096, 256] so each [128, 256] chunk read is contiguous
        wqT_hg = np.ascontiguousarray(wq.T.reshape(HID, 4, 256).transpose(1, 0, 2))
        wkT_hg = np.ascontiguousarray(wk.T.reshape(HID, 4, 256).transpose(1, 0, 2))
        # of-half-major [2, 4096, 512] so each [128, 512] chunk read is contiguous
        woT_ph = np.ascontiguousarray(
            woT_ro.reshape(HID, 2, 512).transpose(1, 0, 2))
        per_core.append({
            "xT": np.ascontiguousarray(x[b].reshape(S, HID).T).astype(_bf16),
            "wqT": wqT_hg.astype(_bf16),
            "wkT": wkT_hg.astype(_bf16),
            "wvT": np.ascontiguousarray(wv.T).astype(_bf16),
            "woT": woT_ph.astype(_bf16),
            "biasq": np.ascontiguousarray(bq_p[hs].T).astype(np.float32),  # [128, 8]
            "biask": np.ascontiguousarray(bk_p[hs].T).astype(np.float32),
            "cosT": np.ascontiguousarray(cosT[b]).astype(_bf16),           # [128, S]
            "sinT": np.ascontiguousarray(sinT[b]).astype(_bf16),
        })
    return shared, per_core, (compute_q, needs_mul, av_js), mask_key


# input order for the run_kernel pytree (list per core)
_IN_NAMES = ["xT", "wqT", "wkT", "wvT", "woT", "pmat", "ident",
             "cosT", "sinT", "emT", "biasq", "biask"]


def _build_kernel_fn(mask_info):
    import concourse.tile as tile
    from concourse import mybir
    dt = mybir.dt
    AF = mybir.ActivationFunctionType
    compute_q, needs_mul, av_js = mask_info

    def kern(tc, outs, ins):
        nc = tc.nc
        (xT_h, wqT_h, wkT_h, wvT_h, woT_h, pmat_h, ident_h,
         cosT_h, sinT_h, emT_h, biasq_h, biask_h) = ins
        out_h = outs[0]

        from contextlib import ExitStack
        with ExitStack() as ctx:
            E = ctx.enter_context
            cpool = E(tc.tile_pool(name="const", bufs=1))
            wpool = E(tc.tile_pool(name="w", bufs=6))
            spool = E(tc.tile_pool(name="s", bufs=2))
            qkpool = E(tc.tile_pool(name="qk", bufs=1))
            vpool = E(tc.tile_pool(name="v", bufs=1))
            prpool = E(tc.tile_pool(name="pr", bufs=2))
            appool = E(tc.tile_pool(name="ap", bufs=2))
            opool = E(tc.tile_pool(name="o", bufs=2))
            agpool = E(tc.tile_pool(name="ag", bufs=6))
            pspool = E(tc.tile_pool(name="ps", bufs=1, space="PSUM"))
            ps2pool = E(tc.tile_pool(name="ps2", bufs=1, space="PSUM"))
            dpool_a1 = E(tc.tile_pool(name="dram_a1", bufs=1, space="DRAM"))
            dpool_a2 = E(tc.tile_pool(name="dram_a2", bufs=1, space="DRAM"))
            dpool_g1 = E(tc.tile_pool(name="dram_g1", bufs=1, space="DRAM"))
            dpool_g2 = E(tc.tile_pool(name="dram_g2", bufs=1, space="DRAM"))

            # constants
            pm = cpool.tile([128, 128], dt.bfloat16, tag="pmat", name="pmat")
            nc.sync.dma_start(pm[:], pmat_h[:, :])
            idn = cpool.tile([128, 128], dt.bfloat16, tag="ident", name="ident")
            nc.sync.dma_start(idn[:], ident_h[:, :])
            bq_sb = cpool.tile([128, HPC], dt.float32, tag="biasq", name="biasq")
            nc.sync.dma_start(bq_sb[:], biasq_h[:, :])
            bk_sb = cpool.tile([128, HPC], dt.float32, tag="biask", name="biask")
            nc.sync.dma_start(bk_sb[:], biask_h[:, :])
            cos_sb = cpool.tile([128, S], dt.bfloat16, tag="cos", name="cos")
            nc.sync.dma_start(cos_sb[:], cosT_h[:, :])
            sin_sb = cpool.tile([128, S], dt.bfloat16, tag="sin", name="sin")
            nc.sync.dma_start(sin_sb[:], sinT_h[:, :])
            em_sb = {}
            for j in range(NKT):
                for qb in needs_mul[j]:
                    t_ = cpool.tile([128, 128], dt.bfloat16, tag=f"em{j}_{qb}",
                                    name=f"em{j}_{qb}")
                    nc.sync.dma_start(t_[:], emT_h[j, :, 128 * qb:128 * qb + 128])
                    em_sb[(j, qb)] = t_

            # token-half-major: rows = 512*qhalf + 128*local_head + hd, cols = 512 tok
            attn_sh1 = dpool_a1.tile([FS, S // 2], dt.bfloat16, tag="attn_sh1",
                                     name="attn_sh1")
            attn_sh2 = dpool_a2.tile([FS, S // 2], dt.bfloat16, tag="attn_sh2",
                                     name="attn_sh2")
            ag1 = dpool_g1.tile([TPG * FS, S // 2], dt.bfloat16, tag="ag1", name="ag1")
            ag2 = dpool_g2.tile([TPG * FS, S // 2], dt.bfloat16, tag="ag2", name="ag2")

            # resident xT chunk tiles; DMAs issued inside the first projection
            # chunk loop so PE starts as soon as chunk 0 lands. The x pool is
            # scoped to the projection phases so its 64KB/partition frees up
            # for the out-projection accumulators.
            xctx = tc.tile_pool(name="x", bufs=1)
            xpool = xctx.__enter__()
            xt = [None] * NC_HID

            # ---- Q and K projections (transposed layout [hd, tok]) ----
            qr, kr = [], []
            for proj, (wT_h, bias_sb, dest) in enumerate(
                    [(wqT_h, bq_sb, qr), (wkT_h, bk_sb, kr)]):
              for hg in range(HPC // 2):
                hpair = [2 * hg, 2 * hg + 1]
                psq = {h: [pspool.tile([128, 512], dt.float32,
                                       tag=f"pb{2*hi+hf}", name=f"pb{2*hi+hf}")
                           for hf in range(2)] for hi, h in enumerate(hpair)}
                for c in range(NC_HID):
                    if xt[c] is None:
                        t_ = xpool.tile([128, S], dt.bfloat16, tag=f"x{c}",
                                        name=f"x{c}")
                        nc.sync.dma_start(t_[:], xT_h[128 * c:128 * c + 128, :])
                        xt[c] = t_
                    wt = wpool.tile([128, 256], dt.bfloat16, tag="wt", name="wt")
                    nc.sync.dma_start(wt[:], wT_h[hg, 128 * c:128 * c + 128, :])
                    for hi, h in enumerate(hpair):
                        lhs = wt[:, 128 * hi:128 * hi + 128]
                        nc.tensor.matmul(psq[h][0][:], lhs, xt[c][:, 0:512],
                                         start=(c == 0), stop=(c == NC_HID - 1))
                        nc.tensor.matmul(psq[h][1][:], lhs, xt[c][:, 512:1024],
                                         start=(c == 0), stop=(c == NC_HID - 1))
                for h in hpair:
                    u = spool.tile([128, S], dt.bfloat16, tag="u", name="u")
                    nc.scalar.activation(u[:, 0:512], psq[h][0][:], AF.Identity,
                                         bias=bias_sb[:, h:h + 1])
                    nc.scalar.activation(u[:, 512:1024], psq[h][1][:], AF.Identity,
                                         bias=bias_sb[:, h:h + 1])
                    psr0 = ps2pool.tile([128, 512], dt.float32, tag="sA", name="sA")
                    nc.tensor.matmul(psr0[:], pm[:], u[:, 0:512],
                                     start=True, stop=True)
                    psr1 = ps2pool.tile([128, 512], dt.float32, tag="sB", name="sB")
                    nc.tensor.matmul(psr1[:], pm[:], u[:, 512:1024],
                                     start=True, stop=True)
                    rot = spool.tile([128, S], dt.bfloat16, tag="rot", name="rot")
                    nc.vector.tensor_copy(rot[:, 0:512], psr0[:])
                    nc.vector.tensor_copy(rot[:, 512:1024], psr1[:])
                    t1 = spool.tile([128, S], dt.bfloat16, tag="t1", name="t1")
                    nc.vector.tensor_mul(t1[:], u[:], cos_sb[:])
                    dst = qkpool.tile([128, S], dt.bfloat16,
                                      tag=f"{'qk'[proj]}r{h}", name=f"{'qk'[proj]}r{h}")
                    nc.vector.tensor_mul(dst[:], rot[:], sin_sb[:])
                    nc.vector.tensor_add(dst[:], dst[:], t1[:])
                    dest.append(dst)

            # ---- V projection (natural layout [tok, feat], 1024 feats) ----
            vt = []
            for t in range(NKT):
                t_ = vpool.tile([128, HPC * VW], dt.bfloat16, tag=f"v{t}", name=f"v{t}")
                vt.append(t_)
            for tg in range(NKT // 2):
                tpair = [2 * tg, 2 * tg + 1]
                psv = {t: [pspool.tile([128, 512], dt.float32,
                                       tag=f"pb{2*ti+tf}", name=f"pb{2*ti+tf}")
                           for tf in range(2)] for ti, t in enumerate(tpair)}
                for c in range(NC_HID):
                    wt = wpool.tile([128, FS], dt.bfloat16, tag="wtv", name="wtv")
                    nc.sync.dma_start(wt[:], wvT_h[128 * c:128 * c + 128, :])
                    for t in tpair:
                        nc.tensor.matmul(psv[t][0][:], xt[c][:, 128 * t:128 * t + 128],
                                         wt[:, 0:512],
                                         start=(c == 0), stop=(c == NC_HID - 1))
                        nc.tensor.matmul(psv[t][1][:], xt[c][:, 128 * t:128 * t + 128],
                                         wt[:, 512:1024],
                                         start=(c == 0), stop=(c == NC_HID - 1))
                for t in tpair:
                    for h in range(HPC):
                        src = psv[t][h // 4][:, 128 * (h % 4):128 * (h % 4) + 128]
                        nc.scalar.copy(vt[t][:, VW * h:VW * h + 128], src)
                        nc.vector.memset(vt[t][:, VW * h + 128:VW * h + 129], 1.0)

            xctx.__exit__(None, None, None)

            # ---- attention; per-key-tile score thunks of head h+1 interleave
            # with per-q-block AV thunks of head h so the PE never waits on
            # the ACT exp stream; AG1 fires after head 3's DMA ----
            ss_ctr = 0

            def make_score_thunks(h):
                nonlocal ss_ctr
                pj = [None] * NKT
                thunks = []
                for j in range(NKT):
                    if not compute_q[j]:
                        continue

                    def th(j=j, h=h):
                        nonlocal ss_ctr
                        p_ = prpool.tile([128, S], dt.bfloat16, tag=f"p{j}",
                                         name=f"p{j}")
                        pj[j] = p_
                        q_lo = 128 * compute_q[j][0]
                        q_hi = 128 * (compute_q[j][-1] + 1)
                        cs = q_lo
                        while cs < q_hi:
                            ce = min(cs + 512, q_hi)
                            stag = "sA" if ss_ctr % 2 == 0 else "sB"
                            ss_ctr += 1
                            pss = ps2pool.tile([128, 512], dt.float32, tag=stag,
                                               name=stag)
                            nc.tensor.matmul(pss[:, 0:ce - cs],
                                             kr[h][:, 128 * j:128 * j + 128],
                                             qr[h][:, cs:ce], start=True, stop=True)
                            nc.scalar.activation(p_[:, cs:ce], pss[:, 0:ce - cs],
                                                 AF.Exp)
                            cs = ce
                        for qb in needs_mul[j]:
                            nc.vector.tensor_mul(p_[:, 128 * qb:128 * qb + 128],
                                                 p_[:, 128 * qb:128 * qb + 128],
                                                 em_sb[(j, qb)][:])
                    thunks.append(th)
                return pj, thunks

            def make_av_thunks(h, pj):
                attnT = appool.tile([128, S], dt.bfloat16, tag="attnT", name="attnT")
                # av/tr psum tiles hold two alternating half-slots each so
                # consecutive i-blocks double-buffer within a single bank;
                # transposes lag the AV matmuls by one i-block so the PE never
                # waits on the reciprocal/scale chain
                psav2 = ps2pool.tile([128, 272], dt.float32, tag="ps_av",
                                     name="ps_av")
                pst2 = ps2pool.tile([128, 256], dt.bfloat16, tag="ps_tr",
                                    name="ps_tr")
                state = {"pending": None}

                def av_block(i):
                    js = av_js[i]
                    ao = 136 * (i % 2)
                    psav = psav2[:, ao:ao + 132]
                    for jx, j in enumerate(js):
                        nc.tensor.matmul(psav[:, 0:129],
                                         pj[j][:, 128 * i:128 * i + 128],
                                         vt[j][:, VW * h:VW * h + 129],
                                         start=(jx == 0), stop=(jx == len(js) - 1))
                    rc = opool.tile([128, 1], dt.float32, tag="rc", name="rc")
                    nc.vector.reciprocal(rc[:], psav[:, 128:129])
                    an = opool.tile([128, 128], dt.bfloat16, tag="an", name="an")
                    nc.vector.tensor_scalar_mul(an[:], psav[:, 0:128], rc[:])
                    if state["pending"] is not None:
                        pi, pan = state["pending"]
                        pst = pst2[:, 128 * (pi % 2):128 * (pi % 2) + 128]
                        nc.tensor.transpose(pst, pan[:], idn[:])
                        nc.vector.tensor_copy(attnT[:, 128 * pi:128 * pi + 128], pst)
                    state["pending"] = (i, an)

                def finish():
                    pi, pan = state["pending"]
                    pst = pst2[:, 128 * (pi % 2):128 * (pi % 2) + 128]
                    nc.tensor.transpose(pst, pan[:], idn[:])
                    nc.vector.tensor_copy(attnT[:, 128 * pi:128 * pi + 128], pst)
                    dst_sh = attn_sh1 if h < 4 else attn_sh2
                    lh = h % 4
                    nc.sync.dma_start(dst_sh[128 * lh:128 * lh + 128, :],
                                      attnT[:, 0:512])
                    nc.sync.dma_start(dst_sh[512 + 128 * lh:512 + 128 * lh + 128, :],
                                      attnT[:, 512:1024])
                    if h == 3:
                        nc.gpsimd.collective_compute(
                            "AllGather", mybir.AluOpType.bypass,
                            replica_groups=[[0, 1, 2, 3], [4, 5, 6, 7]],
                            ins=[attn_sh1.opt()], outs=[ag1.opt()],
                        )
                return [lambda i=i: av_block(i) for i in range(S // 128)] + [finish]

            def run_interleaved(sth, ath):
                # weave the two thunk lists proportionally, scores first
                ns_, na_ = len(sth), len(ath)
                si = ai = 0
                while si < ns_ or ai < na_:
                    if ai >= na_ or (si < ns_ and si * max(na_, 1) <= ai * ns_):
                        sth[si](); si += 1
                    else:
                        ath[ai](); ai += 1

            prev = None
            for h in range(HPC):
                pj_h, sth = make_score_thunks(h)
                ath = make_av_thunks(prev[0], prev[1]) if prev else []
                run_interleaved(sth, ath)
                prev = (h, pj_h)
            for f in make_av_thunks(prev[0], prev[1]):
                f()
            nc.gpsimd.collective_compute(
                "AllGather", mybir.AluOpType.bypass,
                replica_groups=[[0, 1, 2, 3], [4, 5, 6, 7]],
                ins=[attn_sh2.opt()], outs=[ag2.opt()],
            )

            # ---- output projection: token halves x out-feature halves ----
            # contraction chunk c -> (rank rho, local head l): rows in ag buffers
            # at 1024*rho + 512*q + 128*l; all reads contiguous [128, 512].
            # All four passes run their ag1 half (chunks 0-15) first, parking
            # partial sums in SBUF, so ~54us of PE work covers the AG2 flight;
            # then each pass finishes with chunks 16-31 plus the parked half.
            passes = [(0, 0), (0, 1), (1, 0), (1, 1)]
            accpool = E(tc.tile_pool(name="acc", bufs=1))
            accs = {}

            def op_half(q, ph, lo, hi, first_half):
                pso = {t: pspool.tile([128, 512], dt.float32,
                                      tag=f"pb{t}", name=f"pb{t}")
                       for t in range(4)}
                for c in range(lo, hi):
                    src = ag1 if c < 16 else ag2
                    cc = c % 16
                    row = 1024 * (cc // 4) + 512 * q + 128 * (cc % 4)
                    agt = agpool.tile([128, 512], dt.bfloat16, tag="agt",
                                      name="agt")
                    nc.sync.dma_start(agt[:], src[row:row + 128, :])
                    wt = wpool.tile([128, 512], dt.bfloat16, tag="wto", name="wto")
                    nc.scalar.dma_start(wt[:], woT_h[ph, 128 * c:128 * c + 128, :])
                    for t in range(4):
                        nc.tensor.matmul(pso[t][:], agt[:, 128 * t:128 * t + 128],
                                         wt[:], start=(c == lo),
                                         stop=(c == hi - 1))
                if first_half:
                    acc = []
                    for t in range(4):
                        a_ = accpool.tile([128, 512], dt.float32,
                                          tag=f"acc{q}{ph}{t}", name=f"acc{q}{ph}{t}")
                        if t < 2:
                            nc.scalar.copy(a_[:], pso[t][:])
                        else:
                            nc.vector.tensor_copy(a_[:], pso[t][:])
                        acc.append(a_)
                    accs[(q, ph)] = acc
                else:
                    for t in range(4):
                        osb = opool.tile([128, 512], dt.float32, tag="osb",
                                         name="osb")
                        nc.vector.tensor_add(osb[:], pso[t][:],
                                             accs[(q, ph)][t][:])
                        nc.sync.dma_start(
                            out_h[512 * q + 128 * t:512 * q + 128 * t + 128,
                                  512 * ph:512 * ph + 512],
                            osb[:])

            for q, ph in passes:
                op_half(q, ph, 0, 16, True)
            for q, ph in passes:
                op_half(q, ph, 16, NC_HID, False)

    return kern


def _get_program(mask_key, mask_info, shapes):
    if mask_key in _cache:
        return _cache[mask_key]
    import os
    import concourse.tile as tile
    from concourse import bacc, mybir

    trace_sim = bool(os.environ.get("KBENCH_TRACE_SIM"))
    nc = bacc.Bacc("TRN2", target_bir_lowering=False, debug=False,
                   enable_asserts=True, num_devices=NCORES)
    in_aps = []
    for n in _IN_NAMES:
        arr_shape, arr_dt = shapes[n]
        in_aps.append(nc.dram_tensor(
            "in_" + n, list(arr_shape), mybir.dt.from_np(np.dtype(arr_dt)),
            kind="ExternalInput").ap())
    out_ap = nc.dram_tensor("out_sh", [S, OFS], mybir.dt.float32,
                            kind="ExternalOutput").ap()
    kern = _build_kernel_fn(mask_info)
    with tile.TileContext(nc, trace_sim=trace_sim) as tc:
        kern(tc, [out_ap], in_aps)
    nc.compile()
    _cache[mask_key] = nc
    return nc


def kernel(x, Wq, bq, Wk, bk, Wv, bv, Wo, bo, position_ids, attention_mask):
    x = np.asarray(x, dtype=np.float32)
    Wq, bq = np.asarray(Wq, np.float32), np.asarray(bq, np.float32)
    Wk, bk = np.asarray(Wk, np.float32), np.asarray(bk, np.float32)
    Wv, bv = np.asarray(Wv, np.float32), np.asarray(bv, np.float32)
    Wo, bo = np.asarray(Wo, np.float32), np.asarray(bo, np.float32)
    position_ids = np.asarray(position_ids)
    attention_mask = np.asarray(attention_mask, np.float32)

    shared, per_core, mask_info, mask_key = _host_prep(
        x, Wq, bq, Wk, bk, Wv, bv, Wo, bo, position_ids, attention_mask)

    m0 = {**shared, **per_core[0]}
    shapes = {n: (m0[n].shape, m0[n].dtype) for n in _IN_NAMES}
    nc = _get_program(mask_key, mask_info, shapes)

    from concourse import bass2jax
    in_maps = [{"in_" + n: {**shared, **per_core[c]}[n] for n in _IN_NAMES}
               for c in range(NCORES)]
    results = bass2jax.run_bass_via_pjrt(nc, in_maps, n_cores=NCORES)

    out = np.empty((B, S, HID), np.float32)
    for c in range(NCORES):
        b, r = c // TPG, c % TPG
        out[b, :, OFS * r:OFS * r + OFS] = results[c]["out_sh"]

    kernel._last_in_maps = in_maps
    kernel._last_nc = nc

    out = out + (bv @ Wo.T) + bo            # host-folded v/out biases
    return out.astype(np.float32)


def bench(iters=400):
    """Time repeated executions of the last-built program via PJRT.

    Dispatches `iters` executions back-to-back (dropping intermediate result
    references so buffers are freed as they complete) and blocks on the last;
    per-core FIFO execution makes the last result the completion barrier.
    Returns (best_ns, avg_ns) per iteration. Must be called after kernel().
    """
    import time
    import jax
    import jax.numpy as jnp
    from jax.sharding import Mesh, PartitionSpec
    from concourse import bass2jax, mybir
    from jax.experimental.shard_map import shard_map

    nc = kernel._last_nc
    in_maps = kernel._last_in_maps
    bass2jax.install_neuronx_cc_hook()

    in_names, out_names, out_avals, zero_outs = [], [], [], []
    partition_name = nc.partition_id_tensor.name if nc.partition_id_tensor else None
    for alloc in nc.m.functions[0].allocations:
        import concourse.mybir as mb
        if not isinstance(alloc, mb.MemoryLocationSet):
            continue
        name = alloc.memorylocations[0].name
        if alloc.kind == "ExternalInput":
            if name != partition_name:
                in_names.append(name)
        elif alloc.kind == "ExternalOutput":
            shape = tuple(alloc.tensor_shape)
            dtype = mb.dt.np(alloc.dtype)
            out_names.append(name)
            out_avals.append(jax.core.ShapedArray(shape, dtype))
            zero_outs.append(np.zeros(shape, dtype))
    n_params = len(in_names)
    all_in_names = in_names + out_names
    if partition_name is not None:
        all_in_names.append(partition_name)

    def _body(*args):
        operands = list(args)
        if partition_name is not None:
            operands.append(bass2jax.partition_id_tensor())
        outs = bass2jax._bass_exec_p.bind(
            *operands,
            out_avals=tuple(out_avals),
            in_names=tuple(all_in_names),
            out_names=tuple(out_names),
            lowering_input_output_aliases=(),
            sim_require_finite=True,
            sim_require_nnan=True,
            nc=nc,
        )
        return tuple(outs)

    devices = jax.devices()[:NCORES]
    mesh = Mesh(np.asarray(devices), ("core",))
    n_outs = len(out_names)
    in_specs = (PartitionSpec("core"),) * (n_params + n_outs)
    out_specs = (PartitionSpec("core"),) * n_outs
    sharded = jax.jit(shard_map(_body, mesh=mesh, in_specs=in_specs,
                                out_specs=out_specs, check_rep=False),
                      keep_unused=True)
    concat_in = [np.concatenate([np.asarray(in_maps[c][nme]) for c in range(NCORES)],
                                axis=0) for nme in in_names]
    concat_zeros = [np.zeros((NCORES * z.shape[0], *z.shape[1:]), z.dtype)
                    for z in zero_outs]
    from jax.sharding import NamedSharding
    shardings = [NamedSharding(mesh, PartitionSpec("core"))] * (n_params + n_outs)
    dev_in = [jax.device_put(a, s) for a, s in zip(concat_in + concat_zeros, shardings)]
    # warmup (compile)
    out = sharded(*dev_in)
    jax.block_until_ready(out)
    times = []
    for _ in range(3):
        t0 = time.perf_counter()
        last = None
        for _ in range(iters):
            last = sharded(*dev_in)
        jax.block_until_ready(last)
        t1 = time.perf_counter()
        times.append((t1 - t0) / iters)
    return min(times) * 1e9, (sum(times) / len(times)) * 1e9
